# revision 9
# baseline (speedup 1.0000x reference)
"""Trainium2 Bass kernel for nn_MLPFusionLoRA (MoE-routed fused MLP + LoRA).

Sharding: B (batch-per-modality) axis across the 8 NeuronCores — core b gets
sample b of all 4 modalities (the masked routing combine mixes modalities at
fixed b, so each core is self-contained; weights replicated).

v2 over the baseline:
- Per-core modality PERMUTATION + expert-slot compaction: each core reorders
  modalities as [actives..., inactives...] (active set == expert set, the mask
  column is shared). The program is compiled for A = max active slots across
  cores (runtime-derived, cached); LoRA/gate/combine work runs on slots
  0..A-1 only. Inactive/padded slots get zero weights -> exact zeros.
- t1 (gate+a1) and u (a2) matmul clusters are emitted col-group-interleaved
  (tile_position=(0,32k)) so up to A 32-col matmuls stream CONCURRENTLY
  through distinct PE column groups.
- DMA split across the two HWDGE queues (sync: x + fc1-side weights,
  scalar: fc2-side weights + outputs), weight order matches first use.
- x / y DRAM tensors are tile-major so every DMA is fully contiguous.
- Output stored bf16 (halves out-DMA; rel-err impact ~1e-3).

Per-core math (feature-major layout, tokens on the matmul free dim):
  x1_i  = fc1_w @ x_i^T                                  [3072, T]
  t_k   = a1_w[act k] @ x_k^T  (rank 16 in 32-slot)      [128, T]
  w[i,t,e] = E_ie / (D_i + 1e-6*Z_i) * mask[e]*mask[i],  E = exp(gate logits)
  x1_k += B1^T.T @ (t * wexp_k)     <- routing combine folded into one matmul
  h_i   = gelu(x1_i + fc1_b)
  y_i   = fc2_w @ h_i + B2^T.T @ (u * wexp_k) + fc2_b,  u_k = a2_w[act k] @ h_k
"""

from contextlib import ExitStack

import numpy as np
import ml_dtypes

import concourse.bacc as bacc
import concourse.mybir as mybir
import concourse.tile as tile
from concourse import bass_utils
from concourse.bass import ds, ts

F32 = mybir.dt.float32
F32R = mybir.dt.float32r
BF16 = mybir.dt.bfloat16
NPBF = ml_dtypes.bfloat16

M, B, NT, C, H = 4, 8, 1024, 768, 3072
CK, HK = C // 128, H // 128  # 6, 24
T = 256                      # token tile
NTT = NT // T                # 4
AF = mybir.ActivationFunctionType
ALU = mybir.AluOpType

_CACHE = {}


def _build_program(A, nt=NT, gelu=AF.Gelu):
    """A = number of active (modality==expert) slots, 1..4."""
    ntt = nt // T
    nc = bacc.Bacc("TRN2", target_bir_lowering=False, debug=False)

    dp = lambda name, shape, dt: nc.dram_tensor(name, shape, dt, kind="ExternalInput").ap()
    xt = dp("xt", [M, ntt, 128, CK * T], BF16)     # xt[m,tt,p,c*T+t] = x[perm m,b,tt*T+t,128c+p]
    w1 = dp("w1", [HK, 128, CK * 128], BF16)       # j-major: w1[j,p,c*128+q] = fc1_w[128j+q,128c+p]
    w2 = dp("w2", [HK, 128, C], BF16)              # w2[k,p,c] = fc2_w[c,128k+p]
    a1 = dp("a1", [128, CK * 128], BF16)           # [p, c*128 + (32k: A gate cols, +4 ranks)]
    a2 = dp("a2", [128, HK * 128], BF16)           # [p, k*128 + 32s+4+r]
    b1 = dp("b1", [128, H], BF16)                  # b1[32s+4+r,h] = b1_w[act s,h,r]
    b2 = dp("b2", [128, C], BF16)                  # b2[32s+4+r,c] = b2_w[act s,c,r]
    gb = dp("gb", [4, 4], F32)                     # gb[j,k] = gate_b[act k, act j] (Exp bias)
    f1b = dp("f1b", [128, HK], F32)                # fc1_b[128j+p] at [p,j]
    f2b = dp("f2b", [128, CK], F32)                # fc2_b[128j+p] at [p,j]
    mv1 = dp("mv1", [4, 2], F32R)                  # col0: mask values; col1: active indicator
    selm = dp("selm", [4, M * 128], F32R)          # [j, 128k+32j+4+r] = mask[act k]*mask[act j]
    on4 = dp("on4", [1, 4], F32R)                  # ones
    yt = nc.dram_tensor("yt", [M, ntt, CK, 128, T], BF16, kind="ExternalOutput").ap()

    with tile.TileContext(nc) as tc, ExitStack() as ctx:
        wp = ctx.enter_context(tc.tile_pool(name="wts", bufs=1))
        xp = ctx.enter_context(tc.tile_pool(name="xin", bufs=2))
        hp = ctx.enter_context(tc.tile_pool(name="hts", bufs=4))
        sp = ctx.enter_context(tc.tile_pool(name="smal", bufs=2))
        wx = ctx.enter_context(tc.tile_pool(name="wexp", bufs=4))
        syp = ctx.enter_context(tc.tile_pool(name="yout", bufs=2))
        ssp = ctx.enter_context(tc.tile_pool(name="sS", bufs=4))
        pmm = ctx.enter_context(tc.tile_pool(name="pmm", bufs=4, space="PSUM"))
        ptu = ctx.enter_context(tc.tile_pool(name="ptu", bufs=2, space="PSUM"))
        prt = ctx.enter_context(tc.tile_pool(name="prt", bufs=2, space="PSUM"))

        # ---- resident weights ----
        w1s = wp.tile([128, CK * H], BF16)
        w2s = wp.tile([128, HK * C], BF16)
        a1s = wp.tile([128, CK * 128], BF16)
        a2s = wp.tile([128, HK * 128], BF16)
        b1s = wp.tile([128, H], BF16)
        b2s = wp.tile([128, C], BF16)
        gbs = wp.tile([4, 4], F32)
        nc.scalar.dma_start(gbs[:], gb[:])
        f1bs = wp.tile([128, HK], F32)
        f2bs = wp.tile([128, CK], F32)
        mv1s = wp.tile([4, 2], F32R)
        nc.scalar.dma_start(mv1s[:], mv1[:])
        selms = wp.tile([4, M * 128], F32R)
        nc.scalar.dma_start(selms[:], selm[:])
        ones4 = wp.tile([1, 4], F32R)
        nc.scalar.dma_start(ones4[:], on4[:])

        # fc-path modality order: inactive slots first (their fc1 psums have
        # no b1c, so tile-0's routing chain gets covered; fc2's first group
        # similarly buys time for the u->S2 chain)
        fc_order = list(range(A, M)) + list(range(A))

        def load_x(tt):
            xs = xp.tile([128, M * CK * T], BF16, tag="xs", name=f"xs_{tt}")
            xsl = lambda m: xs[:, m * CK * T:(m + 1) * CK * T]
            if tt == 0:
                # Both queues race the preamble: t1 needs a1+x0..x2, the first
                # fc1 slot (inactive) needs x3 + w1[j] as the j-chunks land;
                # f1b must beat the first gelu or the psum ring blocks the PE.
                nc.sync.dma_start(f1bs[:], f1b[:])
                nc.sync.dma_start(xsl(0), xt[0, tt])
                nc.sync.dma_start(xsl(1), xt[1, tt])
                nc.scalar.dma_start(a1s[:], a1[:])
                nc.scalar.dma_start(f2bs[:], f2b[:])
                nc.scalar.dma_start(xsl(2), xt[2, tt])
                nc.scalar.dma_start(xsl(3), xt[3, tt])
                for j in range(HK):
                    nc.sync.dma_start(w1s[:, j * CK * 128:(j + 1) * CK * 128], w1[j])
                nc.scalar.dma_start(b1s[:], b1[:])
                nc.scalar.dma_start(a2s[:], a2[:])
                nc.scalar.dma_start(b2s[:], b2[:])
                for k in range(HK):
                    nc.scalar.dma_start(w2s[:, k * C:(k + 1) * C], w2[k])
            else:
                for m in range(M):
                    nc.sync.dma_start(xsl(m), xt[m, tt])
            st = {"tt": tt, "xs": xs,
                  "xv": lambda m, c, _x=xs: _x[:, (m * CK + c) * T:(m * CK + c + 1) * T]}
            return st

        def emit_t1(st):
            # gate logits ride inside the a1 matmul: block k of a1 carries
            # gate_w[act k] in cols 32k+0..A-1 and a1_w[act k] ranks in cols
            # 32k+4..19, so t1 rows 32k..32k+3 are slot-k gate logits.
            # c-outer / k-inner: adjacent instructions hit distinct PE column
            # groups -> up to A matmuls stream concurrently.
            tt, xv = st["tt"], st["xv"]
            t1 = ptu.tile([128, T], F32, tag="tu", name=f"t1_{tt}")
            for c in range(CK):
                for k in range(A):
                    nc.tensor.matmul(t1[32 * k:32 * k + 32, :],
                                     a1s[:, c * 128 + 32 * k: c * 128 + 32 * k + 32],
                                     xv(k, c), start=(c == 0), stop=(c == CK - 1),
                                     tile_position=(0, 32 * k))
            st["t1"] = t1

        def emit_chains_a(st):
            tt = st["tt"]
            t1s = sp.tile([128, T], F32, tag="t1s", name=f"t1s_{tt}")
            nc.vector.tensor_copy(t1s[:], st["t1"][:])
            st["t1s"] = t1s
            Eis, dens = [], []
            for k in range(A):
                Ei = sp.tile([4, T], F32R, tag="Ei", bufs=4, name=f"Ei_{tt}_{k}")
                nc.scalar.activation(Ei[:], st["t1"][32 * k:32 * k + 4, :],
                                     AF.Exp, bias=gbs[:, k:k + 1])
                Eis.append(Ei)
            for k in range(A):
                dz = prt.tile([1, 2 * T], F32, tag="rt", name=f"dz_{tt}_{k}")
                nc.tensor.matmul(dz[0:1, 0:T], mv1s[:, 0:1], Eis[k][:], start=True, stop=True)
                nc.tensor.matmul(dz[0:1, T:2 * T], mv1s[:, 1:2], Eis[k][:], start=True, stop=True)
                dzs = sp.tile([1, 2 * T], F32, tag="dzs", name=f"dzs_{tt}_{k}")
                nc.vector.tensor_copy(dzs[:], dz[:])
                den = sp.tile([1, T], F32, tag="den", bufs=4, name=f"den_{tt}_{k}")
                nc.vector.scalar_tensor_tensor(den[:], dzs[0:1, T:2 * T], 1e-6,
                                               dzs[0:1, 0:T], ALU.mult, ALU.add)
                dens.append(den)
            st["Eis"], st["dens"] = Eis, dens

        def emit_chains_b(st):
            tt = st["tt"]
            rvs = []
            for k in range(A):
                rv32 = sp.tile([1, T], F32, tag="rv32", name=f"rv32_{tt}_{k}")
                nc.vector.reciprocal_approx_fast(rv32[:], st["dens"][k][:])
                rv = sp.tile([1, T], F32R, tag="rv", bufs=4, name=f"rv_{tt}_{k}")
                nc.vector.tensor_copy(rv[:], rv32[:])
                rvs.append(rv)
            rbs = []
            for k in range(A):
                rb = prt.tile([4, T], F32, tag="rt", name=f"rb_{tt}_{k}")
                nc.tensor.matmul(rb[:], ones4[:], rvs[k][:], start=True, stop=True)
                rbs.append(rb)
            wfs = []
            for k in range(A):
                wf = sp.tile([4, T], F32R, tag="wf", bufs=4, name=f"wf_{tt}_{k}")
                nc.vector.tensor_tensor(wf[:], st["Eis"][k][:], rbs[k][:], ALU.mult)
                wfs.append(wf)
            wexps = []
            for k in range(A):
                wexp_ps = prt.tile([128, T], F32, tag="rt", name=f"wexp_ps_{tt}_{k}")
                nc.tensor.matmul(wexp_ps[:], selms[:, k * 128:(k + 1) * 128],
                                 wfs[k][:], start=True, stop=True)
                wexp = wx.tile([128, T], F32, tag="wexp", name=f"wexp_{tt}_{k}")
                nc.vector.tensor_copy(wexp[:], wexp_ps[:])
                wexps.append(wexp)
            st["wexps"] = wexps

        def emit_S(st):
            tt = st["tt"]
            Ss = []
            for k in range(A):
                S = ssp.tile([128, T], BF16, tag="S1", name=f"S_{tt}_{k}")
                nc.vector.tensor_tensor(S[:], st["t1s"][:], st["wexps"][k][:], ALU.mult)
                Ss.append(S)
            st["Ss"] = Ss

        def emit_fc1(st):
            tt, xv = st["tt"], st["xv"]
            hs = {}
            for i in fc_order:
                hsi = hp.tile([128, HK * T], BF16, tag="hs", name=f"hs_{tt}_{i}")
                hs[i] = hsi
                lora = i < A
                for j in range(HK):
                    x1 = pmm.tile([128, T], F32, tag="mm", name=f"x1_{tt}_{i}_{j}")
                    for c in range(CK):
                        nc.tensor.matmul(x1[:], w1s[:, (j * CK + c) * 128:(j * CK + c + 1) * 128],
                                         xv(i, c), start=(c == 0),
                                         stop=(not lora and c == CK - 1))
                    if lora:
                        nc.tensor.matmul(x1[:], b1s[:, 128 * j:128 * (j + 1)], st["Ss"][i][:],
                                         start=False, stop=True)
                    nc.scalar.activation(hsi[:, j * T:(j + 1) * T], x1[:], gelu,
                                         bias=f1bs[:, j:j + 1])
            st["hs"] = hs

        def emit_u(st):
            # one contiguous col-tiled cluster; j-outer / k-inner so the A
            # 32-col accumulation chains run concurrently in distinct column
            # groups (single tile-mode entry/exit drain for the whole block)
            tt = st["tt"]
            u = ptu.tile([128, T], F32, tag="tu", name=f"u_{tt}")
            hs = st["hs"]
            for j in range(HK):
                for k in range(A):
                    nc.tensor.matmul(u[32 * k:32 * k + 32, :],
                                     a2s[:, j * 128 + 32 * k: j * 128 + 32 * k + 32],
                                     hs[k][:, j * T:(j + 1) * T],
                                     start=(j == 0), stop=(j == HK - 1),
                                     tile_position=(0, 32 * k))
            st["u"] = u

        def emit_us_S2(st):
            tt = st["tt"]
            us = sp.tile([128, T], F32, tag="us", name=f"us_{tt}")
            nc.vector.tensor_copy(us[:], st["u"][:])
            S2s = []
            for k in range(A):
                S2 = ssp.tile([128, T], BF16, tag="S2", name=f"S2_{tt}_{k}")
                nc.vector.tensor_tensor(S2[:], us[:], st["wexps"][k][:], ALU.mult)
                S2s.append(S2)
            st["S2s"] = S2s

        def emit_fc2(st, i_list):
            tt = st["tt"]
            for i in i_list:
                lora = i < A
                for j in range(CK):
                    y = pmm.tile([128, T], F32, tag="mm", name=f"y_{tt}_{i}_{j}")
                    for k in range(HK):
                        nc.tensor.matmul(y[:], w2s[:, k * C + 128 * j: k * C + 128 * (j + 1)],
                                         st["hs"][i][:, k * T:(k + 1) * T],
                                         start=(k == 0),
                                         stop=(not lora and k == HK - 1))
                    if lora:
                        nc.tensor.matmul(y[:], b2s[:, 128 * j:128 * (j + 1)], st["S2s"][i][:],
                                         start=False, stop=True)
                    ysb = syp.tile([128, T], BF16, tag="y", name=f"ysb_{tt}_{i}_{j}")
                    nc.vector.tensor_scalar_add(ysb[:], y[:], f2bs[:, j:j + 1])
                    # alternate output queues so the last tile's writeback
                    # drains at 2x single-ring bandwidth
                    qe = nc.sync if (i * CK + j) % 2 == 0 else nc.scalar
                    qe.dma_start(yt[i, tt, j], ysb[:])

        # fc2 emission groups: first the (or an) inactive slot so the u->S2
        # chain and next tile's routing chains hide under combine-free psums
        g = fc_order
        fc2_groups = [[g[0]], [g[1]], [g[2], g[3]]]

        st = load_x(0)
        emit_t1(st)
        emit_chains_a(st)
        emit_chains_b(st)
        emit_S(st)
        for tt in range(ntt):
            emit_fc1(st)
            nxt = None
            if tt + 1 < ntt:
                nxt = load_x(tt + 1)
            emit_u(st)
            if nxt is not None:
                emit_t1(nxt)  # adjacent to u: same tiled-cluster drain region
            emit_us_S2(st)
            emit_fc2(st, fc2_groups[0])
            if nxt is not None:
                emit_chains_a(nxt)
            emit_fc2(st, fc2_groups[1])
            if nxt is not None:
                emit_chains_b(nxt)
            emit_fc2(st, fc2_groups[2])
            if nxt is not None:
                emit_S(nxt)
                st = nxt

    nc.compile()
    return nc


def _prep_inputs(x, modality_mask, fc1_w, fc1_b, fc2_w, fc2_b, gate_w, gate_b,
                 a1_w, b1_w, a2_w, b2_w):
    """Build the 8 per-core input maps (numpy, host-side layout prep).

    Per-core modality permutation: slots [actives..., inactives...]; the
    active slots double as the expert slots (the mask column is shared).
    Returns (in_maps, perms, A).
    """
    bf = lambda a: np.ascontiguousarray(a).astype(NPBF)
    f32 = lambda a: np.ascontiguousarray(a, dtype=np.float32)

    xm = np.asarray(x, np.float32).reshape(M, B, NT, C)
    # xt_all[b][m,tt,p,c*T+t] = x[m,b,tt*T+t,128c+p]
    xt_all = bf(xm.transpose(1, 0, 3, 2)            # [B,M,C,NT]
                .reshape(B, M, CK, 128, NTT, T)
                .transpose(0, 1, 4, 3, 2, 5)         # [B,M,NTT,128,CK,T]
                .reshape(B, M, NTT, 128, CK * T))

    # j-major w1: w1h[j,p,c*128+q] = fc1_w[128j+q,128c+p]
    w1h = bf(np.asarray(fc1_w, np.float32).T.reshape(CK, 128, HK, 128)
             .transpose(2, 1, 0, 3).reshape(HK, 128, CK * 128))
    w2h = bf(np.asarray(fc2_w, np.float32).T.reshape(HK, 128, C))
    a1t = np.asarray(a1_w, np.float32).transpose(2, 0, 1).reshape(CK, 128, M, 16)
    gwt = np.asarray(gate_w, np.float32).transpose(2, 0, 1).reshape(CK, 128, M, M)
    a2t = np.asarray(a2_w, np.float32).transpose(2, 0, 1).reshape(HK, 128, M, 16)
    b1t = np.asarray(b1_w, np.float32).transpose(0, 2, 1)  # [e, r, h]
    b2t = np.asarray(b2_w, np.float32).transpose(0, 2, 1)  # [e, r, c]
    gbf = np.asarray(gate_b, np.float32)
    f1bh = f32(np.asarray(fc1_b, np.float32).reshape(HK, 128).T)
    f2bh = f32(np.asarray(fc2_b, np.float32).reshape(CK, 128).T)

    maskf = np.asarray(modality_mask, np.float32)  # [M(e), B]
    acts, perms = [], []
    for b in range(B):
        act = [e for e in range(M) if maskf[e, b] != 0.0]
        acts.append(act)
        perms.append(act + [e for e in range(M) if maskf[e, b] == 0.0])
    A = max(1, max(len(a) for a in acts))

    shared = dict(w1=w1h, w2=w2h, f1b=f1bh, f2b=f2bh,
                  on4=np.ones((1, 4), np.float32))

    in_maps = []
    for b in range(B):
        act, perm = acts[b], perms[b]
        mb = maskf[:, b]
        a1p = np.zeros((CK, 128, 128), np.float32)
        a2p = np.zeros((HK, 128, 128), np.float32)
        b1p = np.zeros((128, H), np.float32)
        b2p = np.zeros((128, C), np.float32)
        gbh = np.zeros((4, 4), np.float32)
        mv1 = np.zeros((4, 2), np.float32)
        # Z column all-ones (pad rows included): den >= 4e-6 even on cores
        # with zero active experts, so the reciprocal stays finite
        mv1[:, 1] = 1.0
        selmb = np.zeros((4, M * 128), np.float32)
        for k, e in enumerate(act):
            for j, ej in enumerate(act):
                a1p[:, :, 32 * k + j] = gwt[:, :, e, ej]   # logits for slot j
                gbh[j, k] = gbf[e, ej]
            a1p[:, :, 32 * k + 4:32 * k + 20] = a1t[:, :, e, :]
            a2p[:, :, 32 * k + 4:32 * k + 20] = a2t[:, :, e, :]
            b1p[32 * k + 4:32 * k + 20, :] = b1t[e]
            b2p[32 * k + 4:32 * k + 20, :] = b2t[e]
            mv1[k, 0] = mb[e]
            for j, ej in enumerate(act):
                selmb[j, k * 128 + 32 * j + 4:k * 128 + 32 * j + 20] = mb[e] * mb[ej]
        a1h = a1p.transpose(1, 0, 2).reshape(128, CK * 128)
        a2h = a2p.transpose(1, 0, 2).reshape(128, HK * 128)
        in_maps.append(dict(shared, xt=np.ascontiguousarray(xt_all[b, perm]),
                            a1=bf(a1h), a2=bf(a2h), b1=bf(b1p), b2=bf(b2p),
                            gb=f32(gbh), f1b=f1bh, f2b=f2bh,
                            mv1=f32(mv1), selm=f32(selmb)))
    return in_maps, perms, A


def _run(inputs, trace=False, trace_kwargs=None):
    in_maps, perms, A = _prep_inputs(**inputs)
    if A not in _CACHE:
        _CACHE[A] = _build_program(A)
    nc = _CACHE[A]
    kw = {}
    if trace:
        kw = dict(trace=True, trace_kwargs=trace_kwargs or {})
    res = bass_utils.run_bass_kernel_spmd(nc, in_maps, list(range(B)), **kw)
    # yt[s,tt,c,p,t] -> y[perm[s]*B+b, tt*T+t, 128c+p]
    y = np.empty((M * B, NT, C), np.float32)
    for b in range(B):
        ytb = np.asarray(res.results[b]["yt"], dtype=np.float32)  # [M,NTT,CK,128,T]
        yb = ytb.transpose(0, 1, 4, 2, 3).reshape(M, NT, C)       # [M,tt*T, c*128+p]
        for s in range(M):
            y[perms[b][s] * B + b] = yb[s]
    return y, res


def kernel(**inputs):
    y, _ = _run(inputs)
    return y


# revision 11
# speedup vs baseline: 1.0275x; 1.0275x over previous
"""Trainium2 Bass kernel for nn_MLPFusionLoRA (MoE-routed fused MLP + LoRA).

Sharding: B (batch-per-modality) axis across the 8 NeuronCores — core b gets
sample b of all 4 modalities (the masked routing combine mixes modalities at
fixed b, so each core is self-contained; weights replicated).

v2 over the baseline:
- Per-core modality PERMUTATION + expert-slot compaction: each core reorders
  modalities as [actives..., inactives...] (active set == expert set, the mask
  column is shared). The program is compiled for A = max active slots across
  cores (runtime-derived, cached); LoRA/gate/combine work runs on slots
  0..A-1 only. Inactive/padded slots get zero weights -> exact zeros.
- t1 (gate+a1) and u (a2) matmul clusters are emitted col-group-interleaved
  (tile_position=(0,32k)) so up to A 32-col matmuls stream CONCURRENTLY
  through distinct PE column groups.
- DMA split across the two HWDGE queues (sync: x + fc1-side weights,
  scalar: fc2-side weights + outputs), weight order matches first use.
- x / y DRAM tensors are tile-major so every DMA is fully contiguous.
- Output stored bf16 (halves out-DMA; rel-err impact ~1e-3).

Per-core math (feature-major layout, tokens on the matmul free dim):
  x1_i  = fc1_w @ x_i^T                                  [3072, T]
  t_k   = a1_w[act k] @ x_k^T  (rank 16 in 32-slot)      [128, T]
  w[i,t,e] = E_ie / (D_i + 1e-6*Z_i) * mask[e]*mask[i],  E = exp(gate logits)
  x1_k += B1^T.T @ (t * wexp_k)     <- routing combine folded into one matmul
  h_i   = gelu(x1_i + fc1_b)
  y_i   = fc2_w @ h_i + B2^T.T @ (u * wexp_k) + fc2_b,  u_k = a2_w[act k] @ h_k
"""

from contextlib import ExitStack

import numpy as np
import ml_dtypes

import concourse.bacc as bacc
import concourse.mybir as mybir
import concourse.tile as tile
from concourse import bass_utils
from concourse.bass import ds, ts

F32 = mybir.dt.float32
F32R = mybir.dt.float32r
BF16 = mybir.dt.bfloat16
NPBF = ml_dtypes.bfloat16

M, B, NT, C, H = 4, 8, 1024, 768, 3072
CK, HK = C // 128, H // 128  # 6, 24
T = 256                      # token tile
NTT = NT // T                # 4
AF = mybir.ActivationFunctionType
ALU = mybir.AluOpType

_CACHE = {}


def _build_program(A, nt=NT, gelu=AF.Gelu):
    """A = number of active (modality==expert) slots, 1..4."""
    ntt = nt // T
    nc = bacc.Bacc("TRN2", target_bir_lowering=False, debug=False)

    dp = lambda name, shape, dt: nc.dram_tensor(name, shape, dt, kind="ExternalInput").ap()
    xt = dp("xt", [M, ntt, 128, CK * T], BF16)     # xt[m,tt,p,c*T+t] = x[perm m,b,tt*T+t,128c+p]
    w1 = dp("w1", [HK, 128, CK * 128], BF16)       # j-major: w1[j,p,c*128+q] = fc1_w[128j+q,128c+p]
    w2 = dp("w2", [HK, 128, C], BF16)              # w2[k,p,c] = fc2_w[c,128k+p]
    a1 = dp("a1", [128, CK * 128], BF16)           # [p, c*128 + (32k: A gate cols, +4 ranks)]
    a2 = dp("a2", [128, HK * 128], BF16)           # [p, k*128 + 32s+4+r]
    b1 = dp("b1", [128, H], BF16)                  # b1[32s+4+r,h] = b1_w[act s,h,r]
    b2 = dp("b2", [128, C], BF16)                  # b2[32s+4+r,c] = b2_w[act s,c,r]
    gb = dp("gb", [4, 4], F32)                     # gb[j,k] = gate_b[act k, act j] (Exp bias)
    f1b = dp("f1b", [128, HK], F32)                # fc1_b[128j+p] at [p,j]
    f2b = dp("f2b", [128, CK], F32)                # fc2_b[128j+p] at [p,j]
    mv1 = dp("mv1", [4, 2], F32R)                  # col0: mask values; col1: active indicator
    selm = dp("selm", [4, M * 128], F32R)          # [j, 128k+32j+4+r] = mask[act k]*mask[act j]
    on4 = dp("on4", [1, 4], F32R)                  # ones
    yt = nc.dram_tensor("yt", [M, ntt, CK, 128, T], BF16, kind="ExternalOutput").ap()

    with tile.TileContext(nc) as tc, ExitStack() as ctx:
        wp = ctx.enter_context(tc.tile_pool(name="wts", bufs=1))
        xp = ctx.enter_context(tc.tile_pool(name="xin", bufs=2))
        hp = ctx.enter_context(tc.tile_pool(name="hts", bufs=4))
        sp = ctx.enter_context(tc.tile_pool(name="smal", bufs=2))
        wx = ctx.enter_context(tc.tile_pool(name="wexp", bufs=4))
        syp = ctx.enter_context(tc.tile_pool(name="yout", bufs=2))
        ssp = ctx.enter_context(tc.tile_pool(name="sS", bufs=4))
        pmm = ctx.enter_context(tc.tile_pool(name="pmm", bufs=4, space="PSUM"))
        ptu = ctx.enter_context(tc.tile_pool(name="ptu", bufs=2, space="PSUM"))
        prt = ctx.enter_context(tc.tile_pool(name="prt", bufs=2, space="PSUM"))

        # ---- resident weights ----
        w1s = wp.tile([128, CK * H], BF16)
        w2s = wp.tile([128, HK * C], BF16)
        a1s = wp.tile([128, CK * 128], BF16)
        a2s = wp.tile([128, HK * 128], BF16)
        b1s = wp.tile([128, H], BF16)
        b2s = wp.tile([128, C], BF16)
        # tiny routing tensors ride first on the sync queue: the scalar
        # (ACT) queue must stay free of bulk DMA instructions — a dma_start
        # stalled on ring backpressure blocks every activation behind it
        gbs = wp.tile([4, 4], F32)
        nc.sync.dma_start(gbs[:], gb[:])
        f1bs = wp.tile([128, HK], F32)
        f2bs = wp.tile([128, CK], F32)
        mv1s = wp.tile([4, 2], F32R)
        nc.sync.dma_start(mv1s[:], mv1[:])
        selms = wp.tile([4, M * 128], F32R)
        nc.sync.dma_start(selms[:], selm[:])
        ones4 = wp.tile([1, 4], F32R)
        nc.sync.dma_start(ones4[:], on4[:])

        # fc-path modality order: inactive slots first (their fc1 psums have
        # no b1c, so tile-0's routing chain gets covered; fc2's first group
        # similarly buys time for the u->S2 chain)
        fc_order = list(range(A, M)) + list(range(A))

        def load_x(tt):
            xs = xp.tile([128, M * CK * T], BF16, tag="xs", name=f"xs_{tt}")
            xsl = lambda m: xs[:, m * CK * T:(m + 1) * CK * T]
            if tt == 0:
                # sync: critical-path loads in first-use order (t1 needs
                # a1+x0..x2, first fc1 slot needs x3 + w1[j] per j-chunk,
                # f1b must beat the first gelu or the psum ring blocks).
                # gpsimd (idle engine, software DGE): the fc2-side bulk.
                nc.sync.dma_start(f1bs[:], f1b[:])
                nc.sync.dma_start(a1s[:], a1[:])
                for m in range(M):
                    nc.sync.dma_start(xsl(m), xt[m, tt])
                for j in range(4):
                    nc.sync.dma_start(w1s[:, j * CK * 128:(j + 1) * CK * 128], w1[j])
                nc.sync.dma_start(b1s[:], b1[:])
                for j in range(4, HK):
                    nc.sync.dma_start(w1s[:, j * CK * 128:(j + 1) * CK * 128], w1[j])
                nc.gpsimd.dma_start(f2bs[:], f2b[:])
                nc.gpsimd.dma_start(a2s[:], a2[:])
                nc.gpsimd.dma_start(b2s[:], b2[:])
                for k in range(HK):
                    nc.gpsimd.dma_start(w2s[:, k * C:(k + 1) * C], w2[k])
            else:
                for m in range(M):
                    nc.sync.dma_start(xsl(m), xt[m, tt])
            st = {"tt": tt, "xs": xs,
                  "xv": lambda m, c, _x=xs: _x[:, (m * CK + c) * T:(m * CK + c + 1) * T]}
            return st

        def emit_t1(st):
            # gate logits ride inside the a1 matmul: block k of a1 carries
            # gate_w[act k] in cols 32k+0..A-1 and a1_w[act k] ranks in cols
            # 32k+4..19, so t1 rows 32k..32k+3 are slot-k gate logits.
            # c-outer / k-inner: adjacent instructions hit distinct PE column
            # groups -> up to A matmuls stream concurrently.
            tt, xv = st["tt"], st["xv"]
            t1 = ptu.tile([128, T], F32, tag="tu", name=f"t1_{tt}")
            for c in range(CK):
                for k in range(A):
                    nc.tensor.matmul(t1[32 * k:32 * k + 32, :],
                                     a1s[:, c * 128 + 32 * k: c * 128 + 32 * k + 32],
                                     xv(k, c), start=(c == 0), stop=(c == CK - 1),
                                     tile_position=(0, 32 * k))
            st["t1"] = t1

        def emit_chains_a(st):
            tt = st["tt"]
            t1s = sp.tile([128, T], F32, tag="t1s", name=f"t1s_{tt}")
            nc.vector.tensor_copy(t1s[:], st["t1"][:])
            st["t1s"] = t1s
            Eis, dens = [], []
            for k in range(A):
                Ei = sp.tile([4, T], F32R, tag="Ei", bufs=4, name=f"Ei_{tt}_{k}")
                nc.scalar.activation(Ei[:], st["t1"][32 * k:32 * k + 4, :],
                                     AF.Exp, bias=gbs[:, k:k + 1])
                Eis.append(Ei)
            for k in range(A):
                dz = prt.tile([1, 2 * T], F32, tag="rt", name=f"dz_{tt}_{k}")
                nc.tensor.matmul(dz[0:1, 0:T], mv1s[:, 0:1], Eis[k][:], start=True, stop=True)
                nc.tensor.matmul(dz[0:1, T:2 * T], mv1s[:, 1:2], Eis[k][:], start=True, stop=True)
                dzs = sp.tile([1, 2 * T], F32, tag="dzs", name=f"dzs_{tt}_{k}")
                nc.vector.tensor_copy(dzs[:], dz[:])
                den = sp.tile([1, T], F32, tag="den", bufs=4, name=f"den_{tt}_{k}")
                nc.vector.scalar_tensor_tensor(den[:], dzs[0:1, T:2 * T], 1e-6,
                                               dzs[0:1, 0:T], ALU.mult, ALU.add)
                dens.append(den)
            st["Eis"], st["dens"] = Eis, dens

        def emit_chains_b(st):
            tt = st["tt"]
            rvs = []
            for k in range(A):
                rv32 = sp.tile([1, T], F32, tag="rv32", name=f"rv32_{tt}_{k}")
                nc.vector.reciprocal_approx_fast(rv32[:], st["dens"][k][:])
                rv = sp.tile([1, T], F32R, tag="rv", bufs=4, name=f"rv_{tt}_{k}")
                nc.vector.tensor_copy(rv[:], rv32[:])
                rvs.append(rv)
            rbs = []
            for k in range(A):
                rb = prt.tile([4, T], F32, tag="rt", name=f"rb_{tt}_{k}")
                nc.tensor.matmul(rb[:], ones4[:], rvs[k][:], start=True, stop=True)
                rbs.append(rb)
            wfs = []
            for k in range(A):
                wf = sp.tile([4, T], F32R, tag="wf", bufs=4, name=f"wf_{tt}_{k}")
                nc.vector.tensor_tensor(wf[:], st["Eis"][k][:], rbs[k][:], ALU.mult)
                wfs.append(wf)
            wexps = []
            for k in range(A):
                wexp_ps = prt.tile([128, T], F32, tag="rt", name=f"wexp_ps_{tt}_{k}")
                nc.tensor.matmul(wexp_ps[:], selms[:, k * 128:(k + 1) * 128],
                                 wfs[k][:], start=True, stop=True)
                wexp = wx.tile([128, T], F32, tag="wexp", name=f"wexp_{tt}_{k}")
                nc.vector.tensor_copy(wexp[:], wexp_ps[:])
                wexps.append(wexp)
            st["wexps"] = wexps

        def emit_S(st):
            tt = st["tt"]
            Ss = []
            for k in range(A):
                S = ssp.tile([128, T], BF16, tag="S1", name=f"S_{tt}_{k}")
                nc.vector.tensor_tensor(S[:], st["t1s"][:], st["wexps"][k][:], ALU.mult)
                Ss.append(S)
            st["Ss"] = Ss

        def emit_fc1(st):
            tt, xv = st["tt"], st["xv"]
            hs = {}
            for i in fc_order:
                hsi = hp.tile([128, HK * T], BF16, tag="hs", name=f"hs_{tt}_{i}")
                hs[i] = hsi
                lora = i < A
                for j in range(HK):
                    x1 = pmm.tile([128, T], F32, tag="mm", name=f"x1_{tt}_{i}_{j}")
                    for c in range(CK):
                        nc.tensor.matmul(x1[:], w1s[:, (j * CK + c) * 128:(j * CK + c + 1) * 128],
                                         xv(i, c), start=(c == 0),
                                         stop=(not lora and c == CK - 1))
                    if lora:
                        nc.tensor.matmul(x1[:], b1s[:, 128 * j:128 * (j + 1)], st["Ss"][i][:],
                                         start=False, stop=True)
                    nc.scalar.activation(hsi[:, j * T:(j + 1) * T], x1[:], gelu,
                                         bias=f1bs[:, j:j + 1])
            st["hs"] = hs

        def emit_u(st):
            # one contiguous col-tiled cluster; j-outer / k-inner so the A
            # 32-col accumulation chains run concurrently in distinct column
            # groups (single tile-mode entry/exit drain for the whole block)
            tt = st["tt"]
            u = ptu.tile([128, T], F32, tag="tu", name=f"u_{tt}")
            hs = st["hs"]
            for j in range(HK):
                for k in range(A):
                    nc.tensor.matmul(u[32 * k:32 * k + 32, :],
                                     a2s[:, j * 128 + 32 * k: j * 128 + 32 * k + 32],
                                     hs[k][:, j * T:(j + 1) * T],
                                     start=(j == 0), stop=(j == HK - 1),
                                     tile_position=(0, 32 * k))
            st["u"] = u

        def emit_us_S2(st):
            tt = st["tt"]
            us = sp.tile([128, T], F32, tag="us", name=f"us_{tt}")
            nc.vector.tensor_copy(us[:], st["u"][:])
            S2s = []
            for k in range(A):
                S2 = ssp.tile([128, T], BF16, tag="S2", name=f"S2_{tt}_{k}")
                nc.vector.tensor_tensor(S2[:], us[:], st["wexps"][k][:], ALU.mult)
                S2s.append(S2)
            st["S2s"] = S2s

        def emit_fc2(st, i_list):
            tt = st["tt"]
            for i in i_list:
                lora = i < A
                for j in range(CK):
                    y = pmm.tile([128, T], F32, tag="mm", name=f"y_{tt}_{i}_{j}")
                    for k in range(HK):
                        nc.tensor.matmul(y[:], w2s[:, k * C + 128 * j: k * C + 128 * (j + 1)],
                                         st["hs"][i][:, k * T:(k + 1) * T],
                                         start=(k == 0),
                                         stop=(not lora and k == HK - 1))
                    if lora:
                        nc.tensor.matmul(y[:], b2s[:, 128 * j:128 * (j + 1)], st["S2s"][i][:],
                                         start=False, stop=True)
                    ysb = syp.tile([128, T], BF16, tag="y", name=f"ysb_{tt}_{i}_{j}")
                    nc.vector.tensor_scalar_add(ysb[:], y[:], f2bs[:, j:j + 1])
                    # alternate output queues so the last tile's writeback
                    # drains at 2x single-ring bandwidth
                    qe = nc.sync if (i * CK + j) % 2 == 0 else nc.scalar
                    qe.dma_start(yt[i, tt, j], ysb[:])

        # fc2 emission groups: first the (or an) inactive slot so the u->S2
        # chain and next tile's routing chains hide under combine-free psums
        g = fc_order
        fc2_groups = [[g[0]], [g[1]], [g[2], g[3]]]

        st = load_x(0)
        emit_t1(st)
        emit_chains_a(st)
        emit_chains_b(st)
        emit_S(st)
        for tt in range(ntt):
            emit_fc1(st)
            nxt = None
            if tt + 1 < ntt:
                nxt = load_x(tt + 1)
            emit_u(st)
            if nxt is not None:
                emit_t1(nxt)  # adjacent to u: same tiled-cluster drain region
            emit_us_S2(st)
            emit_fc2(st, fc2_groups[0])
            if nxt is not None:
                emit_chains_a(nxt)
            emit_fc2(st, fc2_groups[1])
            if nxt is not None:
                emit_chains_b(nxt)
            emit_fc2(st, fc2_groups[2])
            if nxt is not None:
                emit_S(nxt)
                st = nxt

    nc.compile()
    return nc


def _prep_inputs(x, modality_mask, fc1_w, fc1_b, fc2_w, fc2_b, gate_w, gate_b,
                 a1_w, b1_w, a2_w, b2_w):
    """Build the 8 per-core input maps (numpy, host-side layout prep).

    Per-core modality permutation: slots [actives..., inactives...]; the
    active slots double as the expert slots (the mask column is shared).
    Returns (in_maps, perms, A).
    """
    bf = lambda a: np.ascontiguousarray(a).astype(NPBF)
    f32 = lambda a: np.ascontiguousarray(a, dtype=np.float32)

    xm = np.asarray(x, np.float32).reshape(M, B, NT, C)
    # xt_all[b][m,tt,p,c*T+t] = x[m,b,tt*T+t,128c+p]
    xt_all = bf(xm.transpose(1, 0, 3, 2)            # [B,M,C,NT]
                .reshape(B, M, CK, 128, NTT, T)
                .transpose(0, 1, 4, 3, 2, 5)         # [B,M,NTT,128,CK,T]
                .reshape(B, M, NTT, 128, CK * T))

    # j-major w1: w1h[j,p,c*128+q] = fc1_w[128j+q,128c+p]
    w1h = bf(np.asarray(fc1_w, np.float32).T.reshape(CK, 128, HK, 128)
             .transpose(2, 1, 0, 3).reshape(HK, 128, CK * 128))
    w2h = bf(np.asarray(fc2_w, np.float32).T.reshape(HK, 128, C))
    a1t = np.asarray(a1_w, np.float32).transpose(2, 0, 1).reshape(CK, 128, M, 16)
    gwt = np.asarray(gate_w, np.float32).transpose(2, 0, 1).reshape(CK, 128, M, M)
    a2t = np.asarray(a2_w, np.float32).transpose(2, 0, 1).reshape(HK, 128, M, 16)
    b1t = np.asarray(b1_w, np.float32).transpose(0, 2, 1)  # [e, r, h]
    b2t = np.asarray(b2_w, np.float32).transpose(0, 2, 1)  # [e, r, c]
    gbf = np.asarray(gate_b, np.float32)
    f1bh = f32(np.asarray(fc1_b, np.float32).reshape(HK, 128).T)
    f2bh = f32(np.asarray(fc2_b, np.float32).reshape(CK, 128).T)

    maskf = np.asarray(modality_mask, np.float32)  # [M(e), B]
    acts, perms = [], []
    for b in range(B):
        act = [e for e in range(M) if maskf[e, b] != 0.0]
        acts.append(act)
        perms.append(act + [e for e in range(M) if maskf[e, b] == 0.0])
    A = max(1, max(len(a) for a in acts))

    shared = dict(w1=w1h, w2=w2h, f1b=f1bh, f2b=f2bh,
                  on4=np.ones((1, 4), np.float32))

    in_maps = []
    for b in range(B):
        act, perm = acts[b], perms[b]
        mb = maskf[:, b]
        a1p = np.zeros((CK, 128, 128), np.float32)
        a2p = np.zeros((HK, 128, 128), np.float32)
        b1p = np.zeros((128, H), np.float32)
        b2p = np.zeros((128, C), np.float32)
        gbh = np.zeros((4, 4), np.float32)
        mv1 = np.zeros((4, 2), np.float32)
        # Z column all-ones (pad rows included): den >= 4e-6 even on cores
        # with zero active experts, so the reciprocal stays finite
        mv1[:, 1] = 1.0
        selmb = np.zeros((4, M * 128), np.float32)
        for k, e in enumerate(act):
            for j, ej in enumerate(act):
                a1p[:, :, 32 * k + j] = gwt[:, :, e, ej]   # logits for slot j
                gbh[j, k] = gbf[e, ej]
            a1p[:, :, 32 * k + 4:32 * k + 20] = a1t[:, :, e, :]
            a2p[:, :, 32 * k + 4:32 * k + 20] = a2t[:, :, e, :]
            b1p[32 * k + 4:32 * k + 20, :] = b1t[e]
            b2p[32 * k + 4:32 * k + 20, :] = b2t[e]
            mv1[k, 0] = mb[e]
            for j, ej in enumerate(act):
                selmb[j, k * 128 + 32 * j + 4:k * 128 + 32 * j + 20] = mb[e] * mb[ej]
        a1h = a1p.transpose(1, 0, 2).reshape(128, CK * 128)
        a2h = a2p.transpose(1, 0, 2).reshape(128, HK * 128)
        in_maps.append(dict(shared, xt=np.ascontiguousarray(xt_all[b, perm]),
                            a1=bf(a1h), a2=bf(a2h), b1=bf(b1p), b2=bf(b2p),
                            gb=f32(gbh), f1b=f1bh, f2b=f2bh,
                            mv1=f32(mv1), selm=f32(selmb)))
    return in_maps, perms, A


def _run(inputs, trace=False, trace_kwargs=None):
    in_maps, perms, A = _prep_inputs(**inputs)
    if A not in _CACHE:
        _CACHE[A] = _build_program(A)
    nc = _CACHE[A]
    kw = {}
    if trace:
        kw = dict(trace=True, trace_kwargs=trace_kwargs or {})
    res = bass_utils.run_bass_kernel_spmd(nc, in_maps, list(range(B)), **kw)
    # yt[s,tt,c,p,t] -> y[perm[s]*B+b, tt*T+t, 128c+p]
    y = np.empty((M * B, NT, C), np.float32)
    for b in range(B):
        ytb = np.asarray(res.results[b]["yt"], dtype=np.float32)  # [M,NTT,CK,128,T]
        yb = ytb.transpose(0, 1, 4, 2, 3).reshape(M, NT, C)       # [M,tt*T, c*128+p]
        for s in range(M):
            y[perms[b][s] * B + b] = yb[s]
    return y, res


def kernel(**inputs):
    y, _ = _run(inputs)
    return y
